# revision 1
# baseline (speedup 1.0000x reference)
"""Trainium2 Bass kernel: 2-layer MLP forward  y = relu(x@W1 + b1) @ W2 + b2.

Shapes: x [262144, 64], W1 [64, 128], b1 [128], W2 [128, 32], b2 [32].
Pure data parallel over 8 NeuronCores, 32768 rows per core.

Per-core dataflow (32 chunks of 1024 rows):
  * Host pre-transposes the x shard to feature-major xt [64, 32768] bf16.
  * xt is DMA'd on a ramped schedule (small segments first for fast
    pipeline fill, 4-chunk segments for steady state), interleaved
    between the SP (HWDGE) and Pool (SWDGE) queues which issue
    concurrently.
  * mm1 (W1 stationary, xt moving): h_ps [128 mid, 1024 rows] PSUM tile.
  * relu+b1 PSUM->SBUF bf16, round-robined between ScalarE (activation
    with per-partition bias) and VectorE (tensor_scalar (h+b1) max 0)
    to balance engine load (GPSIMD has no PSUM port, it cannot help).
  * mm2 (h block stationary, W2 moving): lhsT = h_sb[:, 128j:128j+128],
    rhs = W2 [128, 32] -> y_ps[128 rows, 32]: 32 PE cycles per 128 rows
    instead of 128 (the moving operand is tiny W2, not h).
  * y: each PSUM bank holds 2 chunks (2048 rows) of outputs; DVE adds a
    pre-tiled b2 pattern (tensor_tensor) writing bf16 to SBUF; two banks
    are batched per output DMA, alternating between the SP and Pool
    (SWDGE) queues -- per-queue DMA issue cost is ~2x the transfer time
    in the cost model, so both free queues are used. The final groups
    ship as smaller parallel DMAs to shorten the drain.
  * Chunks are software-pipelined: mm2 for chunk s-1 issues after mm1
    for chunk s, so the PE never waits on the relu engines.
"""

import os
import sys

import numpy as np

if "/opt/trn_rl_repo" not in sys.path:
    sys.path.insert(0, "/opt/trn_rl_repo")

N_CORES = 8
B = 262144
B_C = B // N_CORES  # 32768
N_IN, N_MID, N_OUT = 64, 128, 32
CHUNK = 1024  # rows per chunk (one 2-bank h PSUM tile)
QROWS = 512  # rows per mm1 matmul / y PSUM bank free dim
N_CH = B_C // CHUNK  # 32 chunks
N_YG = B_C // (2 * CHUNK)  # 16 y groups (one PSUM bank per 2 chunks)
# xt DMA schedule: (queue, chunks) in chunk order; SP and Pool run concurrently
X_SCHED = [
    ("sync", 1), ("sync", 1), ("gpsimd", 1), ("gpsimd", 1), ("gpsimd", 4),
    ("sync", 4), ("gpsimd", 4), ("gpsimd", 4), ("gpsimd", 2), ("gpsimd", 2),
    ("gpsimd", 4), ("gpsimd", 2), ("sync", 2),
]

# precision mode: "fast" = bf16 x / bf16 y, "precise" = f32r x / f32 y
MODE = os.environ.get("BASS_MLP_MODE", "fast")
# number of relu tiles (out of N_CH) handled by ACT; rest go to DVE
ACT_RELU = int(os.environ.get("BASS_MLP_ACT_RELU", "22"))
# number of relu tiles handled by Pool/GPSIMD (taken from the DVE share)
# NOTE: Pool/GPSIMD cannot touch PSUM on real HW (BIR verifier rejects
# it at NEFF compile) even though CoreSim accepts and times it. Keep 0.
POOL_RELU = int(os.environ.get("BASS_MLP_POOL_RELU", "0"))
# optional explicit overrides (lists) for tuning; None -> derived defaults
RELU_PATTERN = None  # list of "A"|"V"|"P" of len N_CH
Y_QUEUE = None  # list[str] of len N_YG//2: "sync" | "gpsimd" per pair-DMA
# number of y evacuations (out of N_YG) handled by Pool/GPSIMD; rest on DVE
POOL_Y = int(os.environ.get("BASS_MLP_POOL_Y", "0"))
Y_EVAC = None  # list of "V"|"P" of len N_YG: engine for each y evacuation

_CACHE: dict = {}


def _spread(n_slots: int, n_pick: int) -> list:
    """Evenly spread n_pick True slots over n_slots (Bresenham)."""
    out, err = [], 0
    for _ in range(n_slots):
        err += n_pick
        if err >= n_slots:
            err -= n_slots
            out.append(True)
        else:
            out.append(False)
    return out


def _build_nc(mode: str, act_relu: int, pool_relu: int = POOL_RELU):
    from contextlib import ExitStack

    import concourse.bass as bass  # noqa: F401
    import concourse.tile as tile
    from concourse import bacc, mybir

    f32 = mybir.dt.float32
    bf16 = mybir.dt.bfloat16
    x_dt = bf16 if mode == "fast" else mybir.dt.float32r
    y_dt = bf16 if mode == "fast" else f32
    add = mybir.AluOpType.add
    mx = mybir.AluOpType.max

    y_evac = (
        list(Y_EVAC)
        if Y_EVAC is not None
        else ["P" if x else "V" for x in _spread(N_YG, POOL_Y)]
    )
    if RELU_PATTERN is not None:
        relu_eng = list(RELU_PATTERN)
    elif act_relu == 22:
        # search-tuned default (joint random search, sim 31529)
        relu_eng = list("VAVAAVAAVAAAVAAVAAVAAAVAAVAAVAAA")
    else:
        relu_eng = ["A" if x else "V" for x in _spread(N_CH, act_relu)]
        if pool_relu:
            v_idx = [i for i, e in enumerate(relu_eng) if e == "V"]
            pick = _spread(len(v_idx), pool_relu)
            for i, p in zip(v_idx, pick):
                if p:
                    relu_eng[i] = "P"

    nc = bacc.Bacc(
        "TRN2", target_bir_lowering=False, debug=False, num_devices=N_CORES
    )
    xt_d = nc.dram_tensor("xt", [N_IN, B_C], x_dt, kind="ExternalInput").ap()
    w1_d = nc.dram_tensor("w1", [N_IN, N_MID], x_dt, kind="ExternalInput").ap()
    b1_d = nc.dram_tensor("b1", [N_MID, 1], f32, kind="ExternalInput").ap()
    w2_d = nc.dram_tensor("w2", [N_MID, N_OUT], bf16, kind="ExternalInput").ap()
    b2t_d = nc.dram_tensor("b2t", [N_MID, QROWS], bf16, kind="ExternalInput").ap()
    # pairs of y groups batched per DMA
    y_d = nc.dram_tensor(
        "y", [N_YG // 2, N_MID, 2 * QROWS], y_dt, kind="ExternalOutput"
    ).ap()

    with tile.TileContext(nc) as tc, ExitStack() as ctx:
        consts = ctx.enter_context(tc.tile_pool(name="consts", bufs=1))
        x_pool = ctx.enter_context(tc.tile_pool(name="xp", bufs=5))
        hsb_pool = ctx.enter_context(tc.tile_pool(name="hsb", bufs=5))
        ysb_pool = ctx.enter_context(tc.tile_pool(name="ysb", bufs=3))
        hps_pool = ctx.enter_context(tc.tile_pool(name="hps", bufs=3, space="PSUM"))
        yps_pool = ctx.enter_context(tc.tile_pool(name="yps", bufs=2, space="PSUM"))

        # w1/b1 lead the SP queue (needed by the first chunk); w2/b2t are
        # deferred onto the Pool queue after its first xt segment
        w1_t = consts.tile([N_IN, N_MID], x_dt, name="w1_t")
        nc.sync.dma_start(out=w1_t[:], in_=w1_d)
        w2_t = consts.tile([N_MID, N_OUT], bf16, name="w2_t")
        nc.gpsimd.dma_start(out=w2_t[:], in_=w2_d)
        b1_t = consts.tile([N_MID, 1], f32, name="b1_t")
        nc.sync.dma_start(out=b1_t[:], in_=b1_d)
        b2t_t = consts.tile([N_MID, QROWS], bf16, name="b2t_t")
        nc.gpsimd.dma_start(out=b2t_t[:], in_=b2t_d)

        x_starts = {}
        acc = 0
        for eng, n_chunks in X_SCHED:
            x_starts[acc] = (eng, n_chunks)
            acc += n_chunks
        assert acc == N_CH
        xt_base = 0
        prev = None  # h_sb tile of previous chunk
        y_ps = None
        y_sb = None
        for s in range(N_CH + 1):
            cur = None
            if s < N_CH:
                if s in x_starts:
                    eng, n_chunks = x_starts[s]
                    xt_t = x_pool.tile(
                        [N_IN, n_chunks * CHUNK], x_dt, name="xt_t", tag="xt"
                    )
                    q0 = s * CHUNK
                    getattr(nc, eng).dma_start(
                        out=xt_t[:], in_=xt_d[:, q0 : q0 + n_chunks * CHUNK]
                    )
                    xt_base = s
                h_ps = hps_pool.tile([N_MID, CHUNK], f32, name="h_ps", tag="hps")
                base = (s - xt_base) * CHUNK
                for q in range(CHUNK // QROWS):
                    nc.tensor.matmul(
                        h_ps[:, q * QROWS : (q + 1) * QROWS],
                        w1_t[:],
                        xt_t[:, base + q * QROWS : base + (q + 1) * QROWS],
                        start=True,
                        stop=True,
                    )
                cur = hsb_pool.tile([N_MID, CHUNK], bf16, name="h_sb", tag="hsb")
                if s >= N_CH - 2:
                    nc.scalar.activation(
                        cur[:, :QROWS],
                        h_ps[:, :QROWS],
                        mybir.ActivationFunctionType.Relu,
                        bias=b1_t[:],
                    )
                    nc.vector.tensor_scalar(
                        cur[:, QROWS:], h_ps[:, QROWS:], b1_t[:], 0.0, add, mx
                    )
                elif relu_eng[s] == "A":
                    nc.scalar.activation(
                        cur[:],
                        h_ps[:],
                        mybir.ActivationFunctionType.Relu,
                        bias=b1_t[:],
                    )
                elif relu_eng[s] == "P":
                    nc.gpsimd.tensor_scalar(
                        cur[:], h_ps[:], b1_t[:], 0.0, add, mx
                    )
                else:
                    nc.vector.tensor_scalar(cur[:], h_ps[:], b1_t[:], 0.0, add, mx)
            if s >= 1:
                t = s - 1
                g = t // 2
                if t % 2 == 0:
                    y_ps = yps_pool.tile([N_MID, QROWS], f32, name="y_ps", tag="yps")
                off = (t % 2) * (CHUNK // N_MID) * N_OUT  # 0 or 256
                for j in range(CHUNK // N_MID):  # 8 row-blocks of 128
                    blk = prev[:, j * N_MID : (j + 1) * N_MID]
                    nc.tensor.matmul(
                        y_ps[:, off + j * N_OUT : off + (j + 1) * N_OUT],
                        blk,
                        w2_t[:],
                        start=True,
                        stop=True,
                    )
                if t % 2 == 1:
                    half = g % 2
                    if half == 0:
                        y_sb = ysb_pool.tile(
                            [N_MID, 2 * QROWS], y_dt, name="y_sb", tag="ysb"
                        )
                    y_dst = y_sb[:, half * QROWS : (half + 1) * QROWS]
                    if y_evac[g] == "P":
                        nc.gpsimd.tensor_tensor(y_dst, y_ps[:], b2t_t[:], add)
                    else:
                        nc.vector.tensor_tensor(y_dst, y_ps[:], b2t_t[:], add)
                    if g == N_YG - 1:
                        # final group: two half-DMAs on both queues in parallel
                        hq = QROWS // 2
                        c0 = half * QROWS
                        nc.sync.dma_start(
                            out=y_d[g // 2, :, c0 : c0 + hq],
                            in_=y_sb[:, c0 : c0 + hq],
                        )
                        nc.gpsimd.dma_start(
                            out=y_d[g // 2, :, c0 + hq : c0 + QROWS],
                            in_=y_sb[:, c0 + hq : c0 + QROWS],
                        )
                    elif g == N_YG - 2:
                        # drain: ship each remaining group on its own queue
                        y_eng = nc.sync if g % 2 == 0 else nc.gpsimd
                        y_eng.dma_start(
                            out=y_d[g // 2, :, half * QROWS : (half + 1) * QROWS],
                            in_=y_dst,
                        )
                    elif half == 1:
                        if Y_QUEUE is not None:
                            y_eng = getattr(nc, Y_QUEUE[g // 2])
                        else:
                            y_eng = nc.sync if (g // 2) % 2 == 0 else nc.gpsimd
                        y_eng.dma_start(out=y_d[g // 2], in_=y_sb[:])
            prev = cur

    nc.compile()
    return nc


def _get_nc(mode: str = MODE, act_relu: int = ACT_RELU, pool_relu: int = None):
    if pool_relu is None:
        pool_relu = POOL_RELU
    key = (mode, act_relu, pool_relu)
    if key not in _CACHE:
        _CACHE[key] = _build_nc(mode, act_relu, pool_relu)
    return _CACHE[key]


def _prep_in_maps(x, W1, b1, W2, b2, mode: str = MODE):
    import ml_dtypes

    x_np = ml_dtypes.bfloat16 if mode == "fast" else np.float32
    x = np.ascontiguousarray(x, dtype=np.float32)
    # [8, 64, B_C] feature-major shards
    xt = np.ascontiguousarray(
        x.reshape(N_CORES, B_C, N_IN).transpose(0, 2, 1).astype(x_np)
    )
    w1 = np.ascontiguousarray(W1, dtype=np.float32).astype(x_np)
    w2 = np.ascontiguousarray(W2, dtype=np.float32).astype(ml_dtypes.bfloat16)
    b1c = np.ascontiguousarray(b1, dtype=np.float32).reshape(N_MID, 1)
    b2f = np.asarray(b2, dtype=np.float32)
    # b2 tiled along the free dim: b2t[p, 32*j + o] = b2[o]
    b2t = np.ascontiguousarray(
        np.tile(b2f, (N_MID, QROWS // N_OUT)).astype(ml_dtypes.bfloat16)
    )
    return [
        {"xt": xt[i], "w1": w1, "b1": b1c, "w2": w2, "b2t": b2t}
        for i in range(N_CORES)
    ]


def _unshard(results):
    outs = []
    for i in range(N_CORES):
        yd = np.asarray(results[i]["y"], dtype=np.float32)  # [8, 128, 1024]
        # yd[gp, p, 512*half + 256*u + 32*j + o]
        #   = y[4096*gp + 2048*half + 1024*u + 128*j + p, o]
        y = (
            yd.reshape(N_YG // 2, N_MID, 2, 2, CHUNK // N_MID, N_OUT)
            .transpose(0, 2, 3, 4, 1, 5)
            .reshape(B_C, N_OUT)
        )
        outs.append(y)
    return np.ascontiguousarray(np.concatenate(outs, axis=0))


def run(x, W1, b1, W2, b2, trace=False, mode: str = MODE):
    from concourse.bass_utils import run_bass_kernel_spmd

    nc = _get_nc(mode)
    in_maps = _prep_in_maps(x, W1, b1, W2, b2, mode)
    res = run_bass_kernel_spmd(nc, in_maps, list(range(N_CORES)), trace=trace)
    return _unshard(res.results), res


def kernel(x, W1, b1, W2, b2):
    y, _ = run(x, W1, b1, W2, b2, trace=False)
    return y

